# revision 13
# baseline (speedup 1.0000x reference)
"""Trainium2 Bass kernel for nn_DWNBlock (LRU scan + Lipschitz sandwich MLP).

v2: software-pipelined passes of 2 chunks with an fp8/bf16 precision stack.

Per core (one batch element, x^T channel-major [128, 8192]):
  scan (per chunk of L=512, f32r/bf16):
    Bu via PE (xt-slice stationary, beff moving), pre-scale by lam^-s (DVE,
    bf16 out), shared triangular-ones matmul in bf16 (PE), carry = last
    column of previous chunk's post-scaled H (no extra ops), post-scale
    fused (m + a)*ppos via DVE STT -> bf16 t-tiles, H = t+t on GPSIMD.
  MLP (per pass of 2 chunks, t=1024):
    y = C_re@H_re + (-C_im)@H_im (bf16 PE) + D@x (f32r), drained by DMA;
    G1 f32r; G2/G3 in fp8e4 DoubleRow (K=256 per matmul, 2x PE rate),
    activations quantized to fp8 by the ACT relu drains; G4 bf16 with the
    residual added via an identity matmul; output DMA'd straight from PSUM.
MLP of pass p is interleaved with the scan of pass p+1 so the PE never
waits on activation drains.
"""
import math
import os
import sys

for _p in ('/opt/trn_rl_repo',):
    if _p not in sys.path:
        sys.path.insert(0, _p)

import numpy as np
import ml_dtypes

D = 128          # d_model
N = 128          # d_state
HD = 512         # MLP hidden
T = 8192         # sequence length
B = 8            # batch
L = 512          # scan chunk length
NCORES = 8
SCALE = 1.0
SQRT2 = math.sqrt(2.0)

F8NP = ml_dtypes.float8_e4m3
BF16NP = ml_dtypes.bfloat16

# schedule-tuning knobs (pool ring depths)
TUNE = dict(epool=2, uppool=4, tpool=4, hpool=4, ypool=2, zpool=3, z3pool=8,
            carry=2, bups=2, mps=2, mlpps=2)


# ---------------------------------------------------------------- host prep

def _cayley64(W):
    cout, cin = W.shape
    if cin > cout:
        return _cayley64(W.T).T
    U, V = W[:cin], W[cin:]
    I = np.eye(cin, dtype=W.dtype)
    A = U - U.T + V.T @ V
    iIpA = np.linalg.inv(I + A)
    return np.concatenate([iIpA @ (I - A), -2.0 * V @ iIpA], axis=0)


def _pack_dr(G):
    """Pack a [512, 512] weight into fp8 DoubleRow layout [128, 2048].

    Group g=(j*2+b) holds, for out-columns [128j, +128) and contraction
    k-blocks (2b, 2b+1), the two 128-col weight sets at free offsets
    [g*256 + i*128, +128) for i in {0, 1}.
    """
    out = np.zeros((128, 2048), np.float32)
    for j in range(4):
        for b in range(2):
            g = j * 2 + b
            for i in range(2):
                blk = G[(2 * b + i) * 128:(2 * b + i + 1) * 128,
                        j * 128:(j + 1) * 128]
                out[:, g * 256 + i * 128: g * 256 + (i + 1) * 128] = blk
    return out.astype(F8NP)


def _host_prep(p):
    """Fold all parameters into device constants (float64 -> device dtypes)."""
    f8 = np.float64
    nu_log = p['nu_log'].astype(f8)
    theta_log = p['theta_log'].astype(f8)
    gamma_log = p['gamma_log'].astype(f8)
    lam = np.exp(-np.exp(nu_log)) * np.exp(1j * np.exp(theta_log))   # [N]
    Beff = np.exp(gamma_log)[:, None] * (p['B_re'].astype(f8) + 1j * p['B_im'].astype(f8))
    beff_w = np.concatenate([Beff.real.T, Beff.imag.T], axis=1)      # [D, 2N]

    C = p['C_re'].astype(f8) + 1j * p['C_im'].astype(f8)             # [D, N]
    ytwc = np.concatenate([C.real.T, (-C.imag).T], axis=1)           # [128, 256]
    ytwd = np.ascontiguousarray(p['Dmat'].astype(f8).T)              # [128, 128]

    s = np.arange(L)
    loglam = np.log(lam)
    pneg = np.exp(-s[:, None] * loglam[None, :])    # [L, N] = lam^-s
    ppos = np.exp(s[None, :] * loglam[:, None])     # [N, L] = lam^t'

    def _pk(j):   # s-tile j of pneg, [128, N]
        return pneg[j * 128:(j + 1) * 128, :]
    pneg_pack = np.concatenate([
        _pk(0).real, _pk(0).imag, _pk(1).real, _pk(1).imag,
        _pk(0).imag, _pk(0).real, _pk(1).imag, _pk(1).real,
        _pk(2).real, _pk(2).imag, _pk(3).real, _pk(3).imag,
        _pk(2).imag, _pk(2).real, _pk(3).imag, _pk(3).real,
    ], axis=1)                                       # [128, 2048]

    ppos_pack = np.concatenate([ppos.real, ppos.imag, -ppos.imag], axis=1)  # [128, 1536]

    tri_ones = (np.arange(128)[:, None] <= np.arange(512)[None, :]).astype(f8)

    lamc = np.stack([lam.real, lam.imag, -lam.imag], axis=1)         # [128, 3]

    def _q(Wkey, akey, fout):
        Wd = p[Wkey].astype(f8)
        Q = _cayley64((float(p[akey][0]) / np.linalg.norm(Wd)) * Wd)
        return Q[:, fout:], Q[:, :fout]

    Q1in, Q1out = _q('W1', 'alpha1', HD)
    Q2in, Q2out = _q('W2', 'alpha2', HD)
    Q3in, Q3out = _q('W3', 'alpha3', HD)
    Qlin = _cayley64((float(p['alphal'][0]) / np.linalg.norm(p['Wl'].astype(f8)))
                     * p['Wl'].astype(f8))[:, D:]    # [128, 512]

    e = np.exp
    ps1, ps2, ps3 = p['psi1'].astype(f8), p['psi2'].astype(f8), p['psi3'].astype(f8)
    G1 = SCALE * SCALE * SQRT2 * (Q1in.T * e(-ps1)[None, :])                    # [128, 512]
    G2 = 2.0 * SCALE * (e(ps1)[:, None] * Q1out) @ (Q2in.T * e(-ps2)[None, :])  # [512, 512]
    G3 = 2.0 * SCALE * (e(ps2)[:, None] * Q2out) @ (Q3in.T * e(-ps3)[None, :])  # [512, 512]
    G4 = SQRT2 * SCALE * (e(ps3)[:, None] * Q3out) @ Qlin.T                     # [512, 128]
    g4sb = np.concatenate([G4[k * 128:(k + 1) * 128, :] for k in range(4)],
                          axis=1)                    # [128, 512] block k at cols 128k

    c32 = lambda a: np.ascontiguousarray(a, dtype=np.float32)
    cbf = lambda a: np.ascontiguousarray(np.asarray(a, np.float32).astype(BF16NP))
    return dict(
        beff_w=c32(beff_w),
        ytwc=cbf(ytwc), ytwd=c32(ytwd),
        pneg_pack=c32(pneg_pack), ppos_pack=c32(ppos_pack),
        tri=cbf(tri_ones), lamc=c32(lamc),
        g1=c32(G1), g2p=_pack_dr(G2), g3p=_pack_dr(G3), g4=cbf(g4sb),
        ident=c32(np.eye(128)),
    )


# ---------------------------------------------------------------- device program

def _build_program(t_len, reps=1):
    from contextlib import nullcontext
    from concourse import bacc
    import concourse.mybir as mybir
    from concourse.tile import TileContext

    f32 = mybir.dt.float32
    f32r = mybir.dt.float32r
    bf16 = mybir.dt.bfloat16
    fp8 = mybir.dt.float8e4
    AL = mybir.AluOpType
    ACTF = mybir.ActivationFunctionType
    PM = mybir.MatmulPerfMode
    nchunk = t_len // L
    npass = nchunk // 2

    nc = bacc.Bacc("TRN2", target_bir_lowering=False, debug=False)

    xt_d = nc.dram_tensor("xt", [128, t_len], f32r, kind="ExternalInput").ap()
    beff_d = nc.dram_tensor("beff_w", [128, 256], f32r, kind="ExternalInput").ap()
    ytwc_d = nc.dram_tensor("ytwc", [128, 256], bf16, kind="ExternalInput").ap()
    ytwd_d = nc.dram_tensor("ytwd", [128, 128], f32r, kind="ExternalInput").ap()
    pneg_d = nc.dram_tensor("pneg_pack", [128, 2048], f32, kind="ExternalInput").ap()
    ppos_d = nc.dram_tensor("ppos_pack", [128, 1536], f32, kind="ExternalInput").ap()
    tri_d = nc.dram_tensor("tri", [128, 512], bf16, kind="ExternalInput").ap()
    lamc_d = nc.dram_tensor("lamc", [128, 3], f32, kind="ExternalInput").ap()
    g1_d = nc.dram_tensor("g1", [128, 512], f32r, kind="ExternalInput").ap()
    g2_d = nc.dram_tensor("g2p", [128, 2048], fp8, kind="ExternalInput").ap()
    g3_d = nc.dram_tensor("g3p", [128, 2048], fp8, kind="ExternalInput").ap()
    g4_d = nc.dram_tensor("g4", [128, 512], bf16, kind="ExternalInput").ap()
    id_d = nc.dram_tensor("ident", [128, 128], f32r, kind="ExternalInput").ap()
    out_d = nc.dram_tensor("outT", [128, t_len], f32, kind="ExternalOutput").ap()

    def r(ap):
        return ap if ap.dtype == f32r else ap.bitcast(f32r)

    with TileContext(nc) as tc:
        with (
            tc.tile_pool(name="const", bufs=1) as cpool,
            tc.tile_pool(name="epool", bufs=TUNE["epool"]) as epool,
            tc.tile_pool(name="uppool", bufs=TUNE["uppool"]) as uppool,
            tc.tile_pool(name="tpool", bufs=TUNE["tpool"]) as tpool,
            tc.tile_pool(name="hpool", bufs=TUNE["hpool"]) as hpool,
            tc.tile_pool(name="ypool", bufs=TUNE["ypool"]) as ypool,
            tc.tile_pool(name="zpool", bufs=TUNE["zpool"]) as zpool,
            tc.tile_pool(name="carry", bufs=TUNE["carry"]) as carry_pool,
            tc.tile_pool(name="psum", bufs=2, space="PSUM") as psum,
        ):
            # ---- constants into SBUF
            xt = cpool.tile([128, t_len], f32r, tag="xt")
            for q in range(max(1, t_len // 2048)):
                w = min(2048, t_len)
                nc.sync.dma_start(xt[:, q * w:(q + 1) * w], xt_d[:, q * w:(q + 1) * w])
            beff = cpool.tile([128, 256], f32r, tag="beff")
            nc.sync.dma_start(beff[:], beff_d[:])
            ytwc = cpool.tile([128, 256], bf16, tag="ytwc")
            nc.sync.dma_start(ytwc[:], ytwc_d[:])
            ytwd = cpool.tile([128, 128], f32r, tag="ytwd")
            nc.sync.dma_start(ytwd[:], ytwd_d[:])
            pneg = cpool.tile([128, 2048], f32, tag="pneg")
            nc.sync.dma_start(pneg[:], pneg_d[:])
            ppos = cpool.tile([128, 1536], f32, tag="ppos")
            nc.sync.dma_start(ppos[:], ppos_d[:])
            tri = cpool.tile([128, 512], bf16, tag="tri")
            nc.sync.dma_start(tri[:], tri_d[:])
            lamc = cpool.tile([128, 3], f32, tag="lamc")
            nc.sync.dma_start(lamc[:], lamc_d[:])
            g1 = cpool.tile([128, 512], f32r, tag="g1")
            nc.sync.dma_start(g1[:], g1_d[:])
            g2w = cpool.tile([128, 8, 2, 128], fp8, tag="g2w")
            for k in range(16):
                nc.sync.dma_start(g2w[:, k // 2, k % 2, :],
                                  g2_d[:, k * 128:(k + 1) * 128])
            g3w = cpool.tile([128, 8, 2, 128], fp8, tag="g3w")
            for k in range(16):
                nc.sync.dma_start(g3w[:, k // 2, k % 2, :],
                                  g3_d[:, k * 128:(k + 1) * 128])
            g4 = cpool.tile([128, 512], bf16, tag="g4")
            nc.sync.dma_start(g4[:], g4_d[:])
            ident = cpool.tile([128, 128], f32r, tag="ident")
            nc.sync.dma_start(ident[:], id_d[:])
            zcol = cpool.tile([128, 1], f32, tag="zcol")
            nc.vector.memset(zcol[:], 0.0)

            lam_re, lam_im, lam_imn = lamc[:, 0:1], lamc[:, 1:2], lamc[:, 2:3]
            ppos_re = ppos[:, 0:512]
            ppos_im = ppos[:, 512:1024]
            ppos_imn = ppos[:, 1024:1536]

            st = dict(hcols=(zcol, zcol), prevH=None, prev_t0=0)

            def scan_bu(c):
                """Bu matmuls + pre-scale for chunk c -> two up tiles."""
                t0 = c * L
                ups = []
                for q in range(2):
                    bu = psum.tile([128, 512], f32, tag="bu", bufs=TUNE["bups"],
                                   name=f"bu{c}_{q}")
                    for h2 in range(2):
                        i = 2 * q + h2
                        lhs = xt[:, t0 + i * 128: t0 + (i + 1) * 128]
                        nc.tensor.matmul(bu[:, h2 * 256:(h2 + 1) * 256],
                                         r(lhs), r(beff[:]), start=True, stop=True)
                    e1 = epool.tile([128, 512], f32, tag="e1", name=f"e1_{c}_{q}")
                    e2 = epool.tile([128, 512], f32, tag="e2", name=f"e2_{c}_{q}")
                    pv = pneg[:, q * 1024: q * 1024 + 512]
                    pv_sw = pneg[:, q * 1024 + 512: q * 1024 + 1024]
                    nc.vector.tensor_tensor(e1[:], bu[:], pv, AL.mult)
                    nc.vector.tensor_tensor(e2[:], bu[:], pv_sw, AL.mult)
                    up = uppool.tile([128, 2, 256], bf16, tag="up", name=f"up{c}_{q}")
                    for h2 in range(2):
                        o = h2 * 256
                        nc.vector.tensor_tensor(up[:, h2, 0:128], e1[:, o:o + 128],
                                                e1[:, o + 128:o + 256], AL.subtract)
                        nc.gpsimd.tensor_tensor(up[:, h2, 128:256], e2[:, o:o + 128],
                                                e2[:, o + 128:o + 256], AL.add)
                    ups.append(up)
                return ups

            def scan_tri(c, ups):
                """Triangular scan + carry + post-scale for chunk c -> H tiles."""
                ph_re, ph_im = st['hcols']
                # a = lam * h_prev (tiny per-partition column ops, GPSIMD)
                tmp1 = carry_pool.tile([128, 1], f32, tag="ctmp1", name=f"tmp1_{c}")
                tmp2 = carry_pool.tile([128, 1], f32, tag="ctmp2", name=f"tmp2_{c}")
                tmp3 = carry_pool.tile([128, 1], f32, tag="ctmp3", name=f"tmp3_{c}")
                tmp4 = carry_pool.tile([128, 1], f32, tag="ctmp4", name=f"tmp4_{c}")
                a_re = carry_pool.tile([128, 1], f32, tag="are", name=f"are_{c}")
                a_im = carry_pool.tile([128, 1], f32, tag="aim", name=f"aim_{c}")
                nc.gpsimd.tensor_tensor(tmp1[:], ph_re, lam_re, AL.mult)
                nc.gpsimd.tensor_tensor(tmp3[:], ph_im, lam_imn, AL.mult)
                nc.gpsimd.tensor_tensor(a_re[:], tmp1[:], tmp3[:], AL.add)
                nc.gpsimd.tensor_tensor(tmp2[:], ph_re, lam_im, AL.mult)
                nc.gpsimd.tensor_tensor(tmp4[:], ph_im, lam_re, AL.mult)
                nc.gpsimd.tensor_tensor(a_im[:], tmp2[:], tmp4[:], AL.add)

                m_re = psum.tile([128, 512], f32, tag="m", bufs=TUNE["mps"],
                                 name=f"mre{c}")
                m_im = psum.tile([128, 512], f32, tag="m", bufs=TUNE["mps"],
                                 name=f"mim{c}")
                for j in range(4):
                    up = ups[j // 2]
                    h2 = j % 2
                    width = 512 - 128 * j
                    nc.tensor.matmul(m_re[:, 128 * j:512], up[:, h2, 0:128],
                                     tri[:, 0:width], start=(j == 0), stop=(j == 3))
                for j in range(4):
                    up = ups[j // 2]
                    h2 = j % 2
                    width = 512 - 128 * j
                    nc.tensor.matmul(m_im[:, 128 * j:512], up[:, h2, 128:256],
                                     tri[:, 0:width], start=(j == 0), stop=(j == 3))

                # post-scale: H = (M + a) * ppos  (STT on DVE, adds on GPSIMD)
                t1 = tpool.tile([128, 512], bf16, tag="t1", name=f"t1_{c}")
                t2 = tpool.tile([128, 512], bf16, tag="t2", name=f"t2_{c}")
                t3 = tpool.tile([128, 512], bf16, tag="t3", name=f"t3_{c}")
                t4 = tpool.tile([128, 512], bf16, tag="t4", name=f"t4_{c}")
                nc.vector.scalar_tensor_tensor(t1[:], m_re[:], a_re[:], ppos_re,
                                               AL.add, AL.mult)
                nc.vector.scalar_tensor_tensor(t2[:], m_im[:], a_im[:], ppos_imn,
                                               AL.add, AL.mult)
                nc.vector.scalar_tensor_tensor(t3[:], m_re[:], a_re[:], ppos_im,
                                               AL.add, AL.mult)
                nc.vector.scalar_tensor_tensor(t4[:], m_im[:], a_im[:], ppos_re,
                                               AL.add, AL.mult)
                hre = hpool.tile([128, 512], bf16, tag="hre", name=f"hre{c}")
                him = hpool.tile([128, 512], bf16, tag="him", name=f"him{c}")
                nc.gpsimd.tensor_tensor(hre[:], t1[:], t2[:], AL.add)
                nc.gpsimd.tensor_tensor(him[:], t3[:], t4[:], AL.add)
                st['hcols'] = (hre[:, L - 1:L], him[:, L - 1:L])
                return hre, him

            def mlp_y(env):
                Hs, t0 = env['Hs'], env['t0']
                yp = psum.tile([128, 1024], f32, tag="mlp", bufs=TUNE["mlpps"],
                               name="yp")
                for h in range(2):
                    hre, him = Hs[2 * h], Hs[2 * h + 1]
                    o = h * 512
                    nc.tensor.matmul(yp[:, o:o + 512], ytwc[:, 0:128], hre[:],
                                     start=True, stop=False)
                    nc.tensor.matmul(yp[:, o:o + 512], ytwc[:, 128:256], him[:],
                                     start=False, stop=False)
                    nc.tensor.matmul(yp[:, o:o + 512], r(ytwd[:]),
                                     r(xt[:, t0 + o:t0 + o + 512]),
                                     start=False, stop=True)
                y_sb = ypool.tile([128, 1024], f32r, tag="ysb", name="ysb")
                nc.scalar.copy(y_sb[:, 0:512], yp[:, 0:512])
                nc.scalar.copy(y_sb[:, 512:1024], yp[:, 512:1024])
                env['y_sb'] = y_sb

            def mlp_g1(env):
                y_sb = env['y_sb']
                z1p = [zpool.tile([128, 2, 1024], fp8, tag="z1p", name=f"z1p{b}")
                       for b in range(2)]
                for m in range(4):
                    zp = psum.tile([128, 1024], f32, tag="mlp", bufs=TUNE["mlpps"],
                                   name=f"z1ps{m}")
                    for h in range(2):
                        nc.tensor.matmul(zp[:, h * 512:(h + 1) * 512],
                                         r(g1[:, m * 128:(m + 1) * 128]),
                                         r(y_sb[:, h * 512:(h + 1) * 512]),
                                         start=True, stop=True)
                    nc.scalar.activation(z1p[m // 2][:, m % 2, :], zp[:], ACTF.Relu)
                env['z1p'] = z1p

            def _dr_layer(env, src_key, dst_key, gw, dst_dtype, dst_tag):
                """fp8 DoubleRow 512->512 layer."""
                src = env[src_key]
                if dst_dtype == fp8:
                    dst = [zpool.tile([128, 2, 1024], fp8, tag=dst_tag,
                                      name=f"{dst_tag}{b}") for b in range(2)]
                else:
                    dst = [zpool.tile([128, 1024], bf16, tag=dst_tag,
                                      bufs=TUNE["z3pool"], name=f"{dst_tag}{k}")
                           for k in range(4)]
                for j in range(4):
                    zp = psum.tile([128, 1024], f32, tag="mlp", bufs=TUNE["mlpps"],
                                   name=f"{dst_tag}ps{j}")
                    for b in range(2):
                        w = gw[:, j * 2 + b]          # [128, 2, 128]
                        for h in range(2):
                            outap = zp[:, h * 512:(h + 1) * 512]
                            rhs = src[b][:, :, h * 512:(h + 1) * 512]
                            nc.tensor.matmul(outap, w, rhs,
                                             start=(b == 0), stop=(b == 1),
                                             perf_mode=PM.DoubleRow)
                    if dst_dtype == fp8:
                        nc.scalar.activation(dst[j // 2][:, j % 2, :], zp[:],
                                             ACTF.Relu)
                    else:
                        nc.scalar.activation(dst[j][:], zp[:], ACTF.Relu)
                env[dst_key] = dst

            def mlp_g4(env):
                z3, t0 = env['z3'], env['t0']
                op = psum.tile([128, 1024], f32, tag="mlp", bufs=TUNE["mlpps"],
                               name="outp")
                for k in range(4):
                    for h in range(2):
                        o = h * 512
                        nc.tensor.matmul(op[:, o:o + 512],
                                         g4[:, k * 128:(k + 1) * 128],
                                         z3[k][:, o:o + 512],
                                         start=(k == 0), stop=False)
                for h in range(2):
                    o = h * 512
                    nc.tensor.matmul(op[:, o:o + 512], r(ident[:]),
                                     r(xt[:, t0 + o:t0 + o + 512]),
                                     start=False, stop=True)
                o_sb = ypool.tile([128, 1024], f32, tag="osb", name="osb")
                nc.scalar.copy(o_sb[:, 0:512], op[:, 0:512])
                nc.vector.tensor_scalar_mul(o_sb[:, 512:1024], op[:, 512:1024], 1.0)
                nc.sync.dma_start(out_d[:, t0:t0 + 1024], o_sb[:])

            loop_cm = tc.For_i(0, reps) if reps > 1 else nullcontext()
            with loop_cm:
                st['hcols'] = (zcol, zcol)
                st['prevH'] = None
                for pidx in range(npass):
                    c0, c1 = 2 * pidx, 2 * pidx + 1
                    env = st['prevH']
                    ups0 = scan_bu(c0)
                    if env is not None:
                        mlp_y(env)
                    h0 = scan_tri(c0, ups0)
                    if env is not None:
                        mlp_g1(env)
                    ups1 = scan_bu(c1)
                    h1 = scan_tri(c1, ups1)
                    if env is not None:
                        _dr_layer(env, 'z1p', 'z2p', g2w, fp8, "z2p")
                        _dr_layer(env, 'z2p', 'z3', g3w, bf16, "z3")
                        mlp_g4(env)
                    st['prevH'] = dict(Hs=(h0[0], h0[1], h1[0], h1[1]),
                                       t0=pidx * 1024)
                # tail pass
                env = st['prevH']
                mlp_y(env)
                mlp_g1(env)
                _dr_layer(env, 'z1p', 'z2p', g2w, fp8, "z2p")
                _dr_layer(env, 'z2p', 'z3', g3w, bf16, "z3")
                mlp_g4(env)

    nc.finalize()
    return nc


# ---------------------------------------------------------------- PJRT runner

def _make_runner(nc, n_cores, backend=None):
    import jax
    from jax.sharding import Mesh, PartitionSpec
    from jax.experimental.shard_map import shard_map
    import concourse.mybir as mybir
    from concourse import bass2jax

    bass2jax.install_neuronx_cc_hook()
    assert nc.is_finalized()
    partition_name = nc.partition_id_tensor.name if nc.partition_id_tensor else None

    in_names, out_names, out_avals, zero_shapes = [], [], [], []
    for alloc in nc.m.functions[0].allocations:
        if not isinstance(alloc, mybir.MemoryLocationSet):
            continue
        name = alloc.memorylocations[0].name
        if alloc.kind == "ExternalInput":
            if name != partition_name:
                in_names.append(name)
        elif alloc.kind == "ExternalOutput":
            shape = tuple(alloc.tensor_shape)
            dtype = mybir.dt.np(alloc.dtype)
            out_names.append(name)
            out_avals.append(jax.core.ShapedArray(shape, dtype))
            zero_shapes.append((shape, dtype))
    n_params = len(in_names)
    n_outs = len(out_avals)
    all_in_names = list(in_names) + list(out_names)
    if partition_name is not None:
        all_in_names.append(partition_name)
    donate = tuple(range(n_params, n_params + n_outs))

    def _body(*args):
        operands = list(args)
        if partition_name is not None:
            operands.append(bass2jax.partition_id_tensor())
        outs = bass2jax._bass_exec_p.bind(
            *operands,
            out_avals=tuple(out_avals),
            in_names=tuple(all_in_names),
            out_names=tuple(out_names),
            lowering_input_output_aliases=(),
            sim_require_finite=True,
            sim_require_nnan=True,
            nc=nc,
        )
        return tuple(outs)

    devices = jax.devices(backend)[:n_cores]
    if n_cores == 1:
        fn = jax.jit(_body, donate_argnums=donate, keep_unused=True)
    else:
        mesh = Mesh(np.asarray(devices), ("core",))
        fn = jax.jit(
            shard_map(_body, mesh=mesh,
                      in_specs=(PartitionSpec("core"),) * (n_params + n_outs),
                      out_specs=(PartitionSpec("core"),) * n_outs,
                      check_rep=False),
            donate_argnums=donate, keep_unused=True,
        )

    def run(per_core_inputs):
        import contextlib
        cm = (jax.default_device(devices[0]) if backend is not None
              else contextlib.nullcontext())
        with cm:
            return _run(per_core_inputs)

    def _run(per_core_inputs):
        if n_cores == 1:
            ins = [np.asarray(per_core_inputs[0][n]) for n in in_names]
            zeros = [np.zeros(s, d) for s, d in zero_shapes]
        else:
            ins = [np.concatenate([np.asarray(per_core_inputs[c][n])
                                   for c in range(n_cores)], axis=0) for n in in_names]
            zeros = [np.zeros((n_cores * s[0], *s[1:]), d) for s, d in zero_shapes]
        out_arrs = fn(*ins, *zeros)
        if n_cores == 1:
            return [{name: np.asarray(out_arrs[i]) for i, name in enumerate(out_names)}]
        res = []
        for c in range(n_cores):
            d = {}
            for i, name in enumerate(out_names):
                full = np.asarray(out_arrs[i])
                d[name] = full.reshape(n_cores, *out_avals[i].shape)[c]
            res.append(d)
        return res

    run.fn = fn
    run.in_names = in_names
    run.out_names = out_names
    run.zero_shapes = zero_shapes
    return run


_RUNNER = None


def _get_runner():
    global _RUNNER
    if _RUNNER is None:
        nc = _build_program(T)
        _RUNNER = _make_runner(nc, NCORES)
    return _RUNNER


def kernel(**inputs):
    import time as _time
    global _RUNNER
    p = {k: np.asarray(v) for k, v in inputs.items()}
    consts = _host_prep(p)
    x = p['x'].astype(np.float32)            # [B, T, D]
    per_core = []
    for b in range(B):
        m = dict(consts)
        m['xt'] = np.ascontiguousarray(x[b].T)
        per_core.append(m)
    res = None
    for attempt in range(3):
        try:
            run = _get_runner()
            res = run(per_core)
            break
        except Exception:
            # transient NRT exec faults have been observed on the first
            # execution of a freshly compiled NEFF; rebuild the jitted
            # callable (NEFF comes from the compile cache) and retry.
            _RUNNER = None
            if attempt == 2:
                raise
            _time.sleep(2.0)
    out = np.stack([res[b]['outT'].T for b in range(B)], axis=0)
    return np.ascontiguousarray(out, dtype=np.float32)
